# revision 10
# baseline (speedup 1.0000x reference)
"""Trainium2 Bass kernel for nn_BERTEmbedding_65274912964883.

out[b, l, :] = token_table[seq[b, l]]
             + mean_{g in genres(seq[b, l])} genre_table[g]
             + pos_table[l]

Strategy (8 NeuronCores, SPMD, no collectives):
  - Data-parallel over batch: 256 sequences -> 32 per core (6400 tokens/core).
  - One combined f32 table [VOCAB, 144] replicated per core:
    cols 0..127 token embedding, 128..135 genre ids, 136 count.
  - Per 128-token subtile (token t on partition t % 128): ONE indirect-DMA
    gather of 576B rows. The SWDGE descriptor emission (~9.6ns/row on the
    GpSimd Q7) paces the kernel; all other engines are kept beneath it.
  - genre mean = (one-hot histogram over 21 genres) @ genre_table:
    padded genre slots are remapped out of range (gid + 32*(1-mask));
    the one-hot cube is written in (j, g, s) layout so the s-reduction
    reads contiguously; normalization (x 1/count) is one small DVE op that
    also downcasts to bf16 for the PE; per-subtile PE transposes (base
    partition 0) feed K=21 bf16 matmuls; PSUM->SBUF histogram copies ride
    the otherwise-idle Scalar engine.
  - token + genre + positional adds are group-batched ([128, 512] PSUM
    banks), all f32.
  - positional rows come from a host-prebuilt rotated table (28 rotations,
    f32) -- a single startup DMA, no wrap handling.
  - Macro tiles are tapered [12, 12, 12, 12, 2] so the serial compute tail
    after the last gather is short.
  - Device writes output partition-major [128, N/128, D] f32; host
    un-permutes.
"""

import numpy as np
import ml_dtypes

import concourse.bacc as bacc
import concourse.mybir as mybir
import concourse.tile as tile
from concourse.bass import IndirectOffsetOnAxis
from concourse.bass_utils import run_bass_kernel_spmd

VOCAB = 100000
D = 128
G = 21          # genre ids are in [0, 20]
MAXG = 8
CW = 144        # combined-table row: 128 emb + 8 gid + 1 cnt + 7 pad (f32)
B, L = 256, 200
NCORES = 8
BC = B // NCORES          # sequences per core
N = BC * L                # tokens per core (6400)
SUB = 128                 # tokens per subtile (partition dim)
NSUB = N // SUB           # 50
MACROS = [12, 12, 12, 12, 2]   # subtiles per macro tile (sum = NSUB)
NROT = 25                 # distinct values of (128*i) % 200
NROTX = 28                # extended with 3 duplicates so groups never wrap

F32 = mybir.dt.float32
BF16 = mybir.dt.bfloat16
I32 = mybir.dt.int32

assert sum(MACROS) == NSUB


def emit_core_kernel(tc, seq, ctab, gtab, posrot, giota, iota8, ident, out):
    """Emit the per-core kernel into TileContext `tc`.

    seq    : DRAM [128, NSUB] int32, seq[p, i] = token id of token i*128+p
    ctab   : DRAM [VOCAB, CW] f32 combined table
    gtab   : DRAM [G, D] bf16
    posrot : DRAM [128, NROTX*D] f32, posrot[p, r*D+d] = pos[(128r+p)%200, d]
    giota  : DRAM [128, G] f32, each row = 0..G-1
    iota8  : DRAM [128, MAXG] f32, each row = 0..MAXG-1
    ident  : DRAM [128, 128] bf16 identity
    out    : DRAM [128, NSUB, D] f32, out[p, i, :] = embedding of token i*128+p
    """
    nc = tc.nc
    add = mybir.AluOpType.add
    mult = mybir.AluOpType.mult

    with (
        tc.tile_pool(name="const", bufs=1) as cpool,
        tc.tile_pool(name="work", bufs=2) as wpool,
        tc.tile_pool(name="psum", bufs=2, space="PSUM") as ppool,
    ):
        # --- one-time loads; seq first (gathers depend only on it) ---
        seq_sb = cpool.tile([128, NSUB], I32)
        nc.sync.dma_start(out=seq_sb[:], in_=seq)
        gtab_sb = cpool.tile([G, D], BF16)
        nc.sync.dma_start(out=gtab_sb[:], in_=gtab)
        giota_sb = cpool.tile([128, G], F32)
        nc.sync.dma_start(out=giota_sb[:], in_=giota)
        iota8_sb = cpool.tile([128, MAXG], F32)
        nc.sync.dma_start(out=iota8_sb[:], in_=iota8)
        ident_sb = cpool.tile([128, 128], BF16)
        nc.sync.dma_start(out=ident_sb[:], in_=ident)
        posrot_sb = cpool.tile([128, NROTX * D], F32)
        nc.sync.dma_start(out=posrot_sb[:], in_=posrot)

        # --- main loop over macro tiles ---
        i0 = 0  # global subtile index of the macro's first subtile
        for ksub in MACROS:
            # gather combined rows, one indirect DMA per 128-token subtile
            cg_sb = wpool.tile([128, ksub * CW], F32, tag="cg", bufs=3)
            for j in range(ksub):
                nc.gpsimd.indirect_dma_start(
                    out=cg_sb[:, j * CW:(j + 1) * CW],
                    out_offset=None,
                    in_=ctab,
                    in_offset=IndirectOffsetOnAxis(
                        ap=seq_sb[:, i0 + j:i0 + j + 1], axis=0
                    ),
                )
            cg3 = cg_sb[:].rearrange("p (j c) -> p j c", c=CW)
            gid = cg3[:, :, D:D + MAXG]                # [128, ksub, MAXG]
            cnt = cg3[:, :, D + MAXG:D + MAXG + 1]     # [128, ksub, 1]

            # rec[p, j] = 1 / count
            rec_sb = wpool.tile([128, ksub], F32, tag="rec")
            nc.vector.reciprocal(rec_sb[:], cg3[:, :, D + MAXG])

            # mask[p, j, s] = (s < count[p, j])
            mask_sb = wpool.tile([128, ksub * MAXG], F32, tag="mask")
            m3 = mask_sb[:].rearrange("p (j s) -> p j s", s=MAXG)
            nc.vector.tensor_tensor(
                out=m3,
                in0=iota8_sb[:].unsqueeze(1).broadcast_to([128, ksub, MAXG]),
                in1=cnt.broadcast_to([128, ksub, MAXG]),
                op=mybir.AluOpType.is_lt,
            )
            # shift = 32 * (1 - mask); gidm = gid + shift
            # (padded slots land at >= 32 and never match any genre column)
            shift_sb = wpool.tile([128, ksub * MAXG], F32, tag="shift")
            nc.vector.tensor_scalar(
                out=shift_sb[:], in0=mask_sb[:],
                scalar1=-32.0, scalar2=32.0,
                op0=mult, op1=add,
            )
            gidm_sb = wpool.tile([128, ksub * MAXG], F32, tag="gidm")
            nc.vector.tensor_tensor(
                out=gidm_sb[:].rearrange("p (j s) -> p j s", s=MAXG),
                in0=gid,
                in1=shift_sb[:].rearrange("p (j s) -> p j s", s=MAXG),
                op=add,
            )

            # eq[p, j, g, s] = (gidm[p, j, s] == g)
            # memory layout (j, g, s) so the s-reduction reads contiguously;
            # written with iteration order (j, s, g) via an AP transpose
            eq_sb = wpool.tile([128, ksub * G * MAXG], F32, tag="eq")
            e4 = eq_sb[:].rearrange("p (j g s) -> p j g s", g=G, s=MAXG)
            nc.vector.tensor_tensor(
                out=e4.transpose([0, 1, 3, 2]),
                in0=gidm_sb[:].rearrange("p (j s) -> p j s", s=MAXG)
                    .unsqueeze(3).broadcast_to([128, ksub, MAXG, G]),
                in1=giota_sb[:].unsqueeze(1).unsqueeze(2).broadcast_to(
                    [128, ksub, MAXG, G]
                ),
                op=mybir.AluOpType.is_equal,
            )

            # hist_raw[p, j, g] = sum_s eq[p, j, g, s]
            hist_sb = wpool.tile([128, ksub * G], F32, tag="hist")
            nc.vector.reduce_sum(
                out=hist_sb[:].rearrange("p (j g) -> p j g", g=G),
                in_=e4,
                axis=mybir.AxisListType.X,
            )
            # hist_norm = hist_raw / count   (bf16 for the PE)
            histn_sb = wpool.tile([128, ksub * G], BF16, tag="histn")
            nc.vector.tensor_tensor(
                out=histn_sb[:].rearrange("p (j g) -> p j g", g=G),
                in0=hist_sb[:].rearrange("p (j g) -> p j g", g=G),
                in1=rec_sb[:].unsqueeze(2).broadcast_to([128, ksub, G]),
                op=mult,
            )

            # per-subtile PE transpose of the histogram (base partition 0);
            # PSUM -> SBUF copies ride the otherwise-idle Scalar engine
            histT = []
            for j in range(ksub):
                hT_ps = ppool.tile([G, 128], BF16, tag="hT_ps", bufs=3)
                nc.tensor.transpose(
                    out=hT_ps[:],
                    in_=histn_sb[:, j * G:(j + 1) * G],
                    identity=ident_sb[:],
                )
                hT_sb = wpool.tile([G, 128], BF16, tag="hT_sb", bufs=3)
                nc.scalar.copy(out=hT_sb[:], in_=hT_ps[:])
                histT.append(hT_sb)

            out_sb = wpool.tile([128, ksub * D], F32, tag="outsb")
            for j0 in range(0, ksub, 4):
                ng = min(4, ksub - j0)
                gm_ps = ppool.tile([128, ng * D], F32, tag="gm_ps")
                for k in range(ng):
                    nc.tensor.matmul(
                        out=gm_ps[:, k * D:(k + 1) * D],
                        lhsT=histT[j0 + k][:],
                        rhs=gtab_sb[:],
                        start=True, stop=True,
                    )
                # out = tok + genre_mean + pos   (two group-batched f32 adds)
                oslice = out_sb[:, j0 * D:(j0 + ng) * D]
                nc.vector.tensor_tensor(
                    out=oslice,
                    in0=cg3[:, j0:j0 + ng, 0:D],
                    in1=gm_ps[:],
                    op=add,
                )
                r0 = (i0 + j0) % NROT
                nc.vector.tensor_tensor(
                    out=oslice,
                    in0=oslice,
                    in1=posrot_sb[:, r0 * D:(r0 + ng) * D],
                    op=add,
                )

            # store macro tile: out[p, i0 + j, :] (contiguous per partition)
            nc.sync.dma_start(
                out=out[:, i0:i0 + ksub, :],
                in_=out_sb[:].rearrange("p (j d) -> p j d", d=D),
            )
            i0 += ksub


def build_nc():
    nc = bacc.Bacc("TRN2", target_bir_lowering=False, debug=False)
    seq = nc.dram_tensor("seq", [128, NSUB], I32, kind="ExternalInput").ap()
    ctab = nc.dram_tensor("ctab", [VOCAB, CW], F32, kind="ExternalInput").ap()
    gtab = nc.dram_tensor("gtab", [G, D], BF16, kind="ExternalInput").ap()
    posrot = nc.dram_tensor(
        "posrot", [128, NROTX * D], F32, kind="ExternalInput").ap()
    giota = nc.dram_tensor("giota", [128, G], F32, kind="ExternalInput").ap()
    iota8 = nc.dram_tensor("iota8", [128, MAXG], F32, kind="ExternalInput").ap()
    ident = nc.dram_tensor("ident", [128, 128], BF16, kind="ExternalInput").ap()
    out = nc.dram_tensor("out", [128, NSUB, D], F32, kind="ExternalOutput").ap()

    with tile.TileContext(nc) as tc:
        emit_core_kernel(tc, seq, ctab, gtab, posrot, giota, iota8, ident, out)
    nc.compile()
    return nc


_NC_CACHE = None


def _get_nc():
    global _NC_CACHE
    if _NC_CACHE is None:
        _NC_CACHE = build_nc()
    return _NC_CACHE


def make_ctab(token_table, token_genre_ids, genre_counts):
    ctab = np.zeros((VOCAB, CW), dtype=np.float32)
    ctab[:, 0:D] = np.asarray(token_table, dtype=np.float32)
    ctab[:, D:D + MAXG] = np.asarray(token_genre_ids, dtype=np.float32)
    ctab[:, D + MAXG] = np.asarray(genre_counts, dtype=np.float32)
    return ctab


def make_posrot(pos_table):
    pos = np.asarray(pos_table, dtype=np.float32)
    pr = np.zeros((128, NROTX * D), dtype=np.float32)
    p = np.arange(128)
    for r in range(NROTX):
        pr[:, r * D:(r + 1) * D] = pos[(128 * r + p) % L, :]
    return pr


def prep_host_inputs(sequence, token_table, genre_table, pos_table,
                     token_genre_ids, genre_counts):
    """Host-side sharding / layout prep. Returns in_maps for the 8 cores."""
    seq = np.ascontiguousarray(np.asarray(sequence).astype(np.int32)).reshape(B, L)
    ctab = make_ctab(token_table, token_genre_ids, genre_counts)
    gtab = np.asarray(genre_table, dtype=np.float32).astype(ml_dtypes.bfloat16)
    posrot = make_posrot(pos_table)

    giota = np.broadcast_to(np.arange(G, dtype=np.float32), (128, G)).copy()
    iota8 = np.broadcast_to(
        np.arange(MAXG, dtype=np.float32), (128, MAXG)).copy()
    ident = np.eye(128, dtype=np.float32).astype(ml_dtypes.bfloat16)

    in_maps = []
    for c in range(NCORES):
        seq_core = seq[c * BC:(c + 1) * BC].reshape(N)
        # device layout: seq_dev[p, i] = seq_core[i*128 + p]
        seq_dev = np.ascontiguousarray(seq_core.reshape(NSUB, 128).T)
        in_maps.append({
            "seq": seq_dev,
            "ctab": ctab,
            "gtab": gtab,
            "posrot": posrot,
            "giota": giota,
            "iota8": iota8,
            "ident": ident,
        })
    return in_maps


def postprocess(results):
    """Un-permute per-core outputs and concatenate to [B, L, D]."""
    outs = []
    for c in range(NCORES):
        o = results[c]["out"]  # [128, NSUB, D]
        outs.append(np.ascontiguousarray(o.transpose(1, 0, 2)).reshape(BC, L, D))
    return np.concatenate(outs, axis=0)


def kernel(sequence, token_table, genre_table, pos_table, token_genre_ids,
           genre_counts):
    nc = _get_nc()
    in_maps = prep_host_inputs(sequence, token_table, genre_table, pos_table,
                               token_genre_ids, genre_counts)
    res = run_bass_kernel_spmd(nc, in_maps, core_ids=list(range(NCORES)))
    return postprocess(res.results)


# revision 12
# speedup vs baseline: 1.0345x; 1.0345x over previous
"""Trainium2 Bass kernel for nn_BERTEmbedding_65274912964883.

out[b, l, :] = token_table[seq[b, l]]
             + mean_{g in genres(seq[b, l])} genre_table[g]
             + pos_table[l]

Strategy (8 NeuronCores, SPMD, no collectives):
  - Data-parallel over batch: 256 sequences -> 32 per core (6400 tokens/core).
  - One combined f32 table [VOCAB, 144] replicated per core:
    cols 0..127 token embedding, 128..135 genre ids, 136 count.
  - Per 128-token subtile (token t on partition t % 128): ONE indirect-DMA
    gather of 576B rows. The SWDGE descriptor emission (~9.6ns/row on the
    GpSimd Q7) paces the kernel; all other engines are kept beneath it.
  - genre mean = (one-hot histogram over 21 genres) @ genre_table:
    padded genre slots are remapped out of range (gid + 32*(1-mask));
    the one-hot cube is written in (j, g, s) layout so the s-reduction
    reads contiguously; normalization (x 1/count) is one small DVE op that
    also downcasts to bf16 for the PE; per-subtile PE transposes (base
    partition 0) feed K=21 bf16 matmuls; PSUM->SBUF histogram copies ride
    the otherwise-idle Scalar engine.
  - token + genre + positional adds are group-batched ([128, 512] PSUM
    banks), all f32.
  - positional rows come from a host-prebuilt rotated table (28 rotations,
    f32) -- a single startup DMA, no wrap handling.
  - Macro tiles are tapered [12, 12, 12, 12, 2] so the serial compute tail
    after the last gather is short.
  - Device writes output partition-major [128, N/128, D] f32; host
    un-permutes.
"""

import numpy as np
import ml_dtypes

import concourse.bacc as bacc
import concourse.mybir as mybir
import concourse.tile as tile
from concourse.bass import IndirectOffsetOnAxis
from concourse.bass_utils import run_bass_kernel_spmd

VOCAB = 100000
D = 128
G = 21          # genre ids are in [0, 20]
MAXG = 8
CW = 144        # combined-table row: 128 emb + 8 gid + 1 cnt + 7 pad (f32)
B, L = 256, 200
NCORES = 8
BC = B // NCORES          # sequences per core
N = BC * L                # tokens per core (6400)
SUB = 128                 # tokens per subtile (partition dim)
NSUB = N // SUB           # 50
MACROS = [12, 12, 12, 12, 2]   # subtiles per macro tile (sum = NSUB)
NROT = 25                 # distinct values of (128*i) % 200
NROTX = 28                # extended with 3 duplicates so groups never wrap

F32 = mybir.dt.float32
BF16 = mybir.dt.bfloat16
I32 = mybir.dt.int32

assert sum(MACROS) == NSUB


def emit_core_kernel(tc, seq, ctab, gtab, posrot, giota, iota8, ident, out):
    """Emit the per-core kernel into TileContext `tc`.

    seq    : DRAM [128, NSUB] int32, seq[p, i] = token id of token i*128+p
    ctab   : DRAM [VOCAB, CW] f32 combined table
    gtab   : DRAM [G, D] bf16
    posrot : DRAM [128, NROTX*D] f32, posrot[p, r*D+d] = pos[(128r+p)%200, d]
    giota  : DRAM [128, G] f32, each row = 0..G-1
    iota8  : DRAM [128, MAXG] f32, each row = 0..MAXG-1
    ident  : DRAM [128, 128] bf16 identity
    out    : DRAM [128, NSUB, D] f32, out[p, i, :] = embedding of token i*128+p
    """
    nc = tc.nc
    add = mybir.AluOpType.add
    mult = mybir.AluOpType.mult

    with (
        tc.tile_pool(name="const", bufs=1) as cpool,
        tc.tile_pool(name="work", bufs=2) as wpool,
        tc.tile_pool(name="psum", bufs=2, space="PSUM") as ppool,
    ):
        # --- one-time loads; seq first (gathers depend only on it) ---
        seq_sb = cpool.tile([128, NSUB], I32)
        nc.sync.dma_start(out=seq_sb[:], in_=seq)
        gtab_sb = cpool.tile([G, D], BF16)
        nc.sync.dma_start(out=gtab_sb[:], in_=gtab)
        giota_sb = cpool.tile([128, G], F32)
        nc.sync.dma_start(out=giota_sb[:], in_=giota)
        iota8_sb = cpool.tile([128, MAXG], F32)
        nc.sync.dma_start(out=iota8_sb[:], in_=iota8)
        ident_sb = cpool.tile([128, 128], BF16)
        nc.sync.dma_start(out=ident_sb[:], in_=ident)
        posrot_sb = cpool.tile([128, NROTX * D], F32)
        nc.sync.dma_start(out=posrot_sb[:], in_=posrot)

        # --- main loop over macro tiles ---
        i0 = 0  # global subtile index of the macro's first subtile
        for ksub in MACROS:
            # gather combined rows, one indirect DMA per 128-token subtile
            cg_sb = wpool.tile([128, ksub * CW], F32, tag="cg", bufs=4)
            for j in range(ksub):
                nc.gpsimd.indirect_dma_start(
                    out=cg_sb[:, j * CW:(j + 1) * CW],
                    out_offset=None,
                    in_=ctab,
                    in_offset=IndirectOffsetOnAxis(
                        ap=seq_sb[:, i0 + j:i0 + j + 1], axis=0
                    ),
                )
            cg3 = cg_sb[:].rearrange("p (j c) -> p j c", c=CW)
            gid = cg3[:, :, D:D + MAXG]                # [128, ksub, MAXG]
            cnt = cg3[:, :, D + MAXG:D + MAXG + 1]     # [128, ksub, 1]

            # rec[p, j] = 1 / count
            rec_sb = wpool.tile([128, ksub], F32, tag="rec")
            nc.vector.reciprocal(rec_sb[:], cg3[:, :, D + MAXG])

            # mask[p, j, s] = (s < count[p, j])
            mask_sb = wpool.tile([128, ksub * MAXG], F32, tag="mask")
            m3 = mask_sb[:].rearrange("p (j s) -> p j s", s=MAXG)
            nc.vector.tensor_tensor(
                out=m3,
                in0=iota8_sb[:].unsqueeze(1).broadcast_to([128, ksub, MAXG]),
                in1=cnt.broadcast_to([128, ksub, MAXG]),
                op=mybir.AluOpType.is_lt,
            )
            # shift = 32 * (1 - mask); gidm = gid + shift
            # (padded slots land at >= 32 and never match any genre column)
            shift_sb = wpool.tile([128, ksub * MAXG], F32, tag="shift")
            nc.vector.tensor_scalar(
                out=shift_sb[:], in0=mask_sb[:],
                scalar1=-32.0, scalar2=32.0,
                op0=mult, op1=add,
            )
            gidm_sb = wpool.tile([128, ksub * MAXG], F32, tag="gidm")
            nc.vector.tensor_tensor(
                out=gidm_sb[:].rearrange("p (j s) -> p j s", s=MAXG),
                in0=gid,
                in1=shift_sb[:].rearrange("p (j s) -> p j s", s=MAXG),
                op=add,
            )

            # eq[p, j, s, g] = (gidm[p, j, s] == g)   (contiguous write)
            eq_sb = wpool.tile([128, ksub * MAXG * G], F32, tag="eq")
            e4 = eq_sb[:].rearrange("p (j s g) -> p j s g", s=MAXG, g=G)
            nc.vector.tensor_tensor(
                out=e4,
                in0=gidm_sb[:].rearrange("p (j s) -> p j s", s=MAXG)
                    .unsqueeze(3).broadcast_to([128, ksub, MAXG, G]),
                in1=giota_sb[:].unsqueeze(1).unsqueeze(2).broadcast_to(
                    [128, ksub, MAXG, G]
                ),
                op=mybir.AluOpType.is_equal,
            )

            # hist_raw[p, j, g] = sum_s eq[p, j, s, g]  (strided s-reduction)
            hist_sb = wpool.tile([128, ksub * G], F32, tag="hist")
            nc.vector.reduce_sum(
                out=hist_sb[:].rearrange("p (j g) -> p j g", g=G),
                in_=eq_sb[:].rearrange("p (j s g) -> p j g s", s=MAXG, g=G),
                axis=mybir.AxisListType.X,
            )
            # hist_norm = hist_raw / count   (bf16 for the PE)
            histn_sb = wpool.tile([128, ksub * G], BF16, tag="histn")
            nc.vector.tensor_tensor(
                out=histn_sb[:].rearrange("p (j g) -> p j g", g=G),
                in0=hist_sb[:].rearrange("p (j g) -> p j g", g=G),
                in1=rec_sb[:].unsqueeze(2).broadcast_to([128, ksub, G]),
                op=mult,
            )

            # per-subtile PE transpose of the histogram (base partition 0);
            # PSUM -> SBUF copies ride the otherwise-idle Scalar engine
            histT = []
            for j in range(ksub):
                hT_ps = ppool.tile([G, 128], BF16, tag="hT_ps", bufs=3)
                nc.tensor.transpose(
                    out=hT_ps[:],
                    in_=histn_sb[:, j * G:(j + 1) * G],
                    identity=ident_sb[:],
                )
                hT_sb = wpool.tile([G, 128], BF16, tag="hT_sb", bufs=3)
                nc.scalar.copy(out=hT_sb[:], in_=hT_ps[:])
                histT.append(hT_sb)

            out_sb = wpool.tile([128, ksub * D], F32, tag="outsb")
            for j0 in range(0, ksub, 4):
                ng = min(4, ksub - j0)
                gm_ps = ppool.tile([128, ng * D], F32, tag="gm_ps")
                for k in range(ng):
                    nc.tensor.matmul(
                        out=gm_ps[:, k * D:(k + 1) * D],
                        lhsT=histT[j0 + k][:],
                        rhs=gtab_sb[:],
                        start=True, stop=True,
                    )
                # out = tok + genre_mean + pos   (two group-batched f32 adds)
                oslice = out_sb[:, j0 * D:(j0 + ng) * D]
                nc.vector.tensor_tensor(
                    out=oslice,
                    in0=cg3[:, j0:j0 + ng, 0:D],
                    in1=gm_ps[:],
                    op=add,
                )
                r0 = (i0 + j0) % NROT
                nc.vector.tensor_tensor(
                    out=oslice,
                    in0=oslice,
                    in1=posrot_sb[:, r0 * D:(r0 + ng) * D],
                    op=add,
                )

            # store macro tile: out[p, i0 + j, :] (contiguous per partition)
            nc.sync.dma_start(
                out=out[:, i0:i0 + ksub, :],
                in_=out_sb[:].rearrange("p (j d) -> p j d", d=D),
            )
            i0 += ksub


def build_nc():
    nc = bacc.Bacc("TRN2", target_bir_lowering=False, debug=False)
    seq = nc.dram_tensor("seq", [128, NSUB], I32, kind="ExternalInput").ap()
    ctab = nc.dram_tensor("ctab", [VOCAB, CW], F32, kind="ExternalInput").ap()
    gtab = nc.dram_tensor("gtab", [G, D], BF16, kind="ExternalInput").ap()
    posrot = nc.dram_tensor(
        "posrot", [128, NROTX * D], F32, kind="ExternalInput").ap()
    giota = nc.dram_tensor("giota", [128, G], F32, kind="ExternalInput").ap()
    iota8 = nc.dram_tensor("iota8", [128, MAXG], F32, kind="ExternalInput").ap()
    ident = nc.dram_tensor("ident", [128, 128], BF16, kind="ExternalInput").ap()
    out = nc.dram_tensor("out", [128, NSUB, D], F32, kind="ExternalOutput").ap()

    with tile.TileContext(nc) as tc:
        emit_core_kernel(tc, seq, ctab, gtab, posrot, giota, iota8, ident, out)
    nc.compile()
    return nc


_NC_CACHE = None


def _get_nc():
    global _NC_CACHE
    if _NC_CACHE is None:
        _NC_CACHE = build_nc()
    return _NC_CACHE


def make_ctab(token_table, token_genre_ids, genre_counts):
    ctab = np.zeros((VOCAB, CW), dtype=np.float32)
    ctab[:, 0:D] = np.asarray(token_table, dtype=np.float32)
    ctab[:, D:D + MAXG] = np.asarray(token_genre_ids, dtype=np.float32)
    ctab[:, D + MAXG] = np.asarray(genre_counts, dtype=np.float32)
    return ctab


def make_posrot(pos_table):
    pos = np.asarray(pos_table, dtype=np.float32)
    pr = np.zeros((128, NROTX * D), dtype=np.float32)
    p = np.arange(128)
    for r in range(NROTX):
        pr[:, r * D:(r + 1) * D] = pos[(128 * r + p) % L, :]
    return pr


def prep_host_inputs(sequence, token_table, genre_table, pos_table,
                     token_genre_ids, genre_counts):
    """Host-side sharding / layout prep. Returns in_maps for the 8 cores."""
    seq = np.ascontiguousarray(np.asarray(sequence).astype(np.int32)).reshape(B, L)
    ctab = make_ctab(token_table, token_genre_ids, genre_counts)
    gtab = np.asarray(genre_table, dtype=np.float32).astype(ml_dtypes.bfloat16)
    posrot = make_posrot(pos_table)

    giota = np.broadcast_to(np.arange(G, dtype=np.float32), (128, G)).copy()
    iota8 = np.broadcast_to(
        np.arange(MAXG, dtype=np.float32), (128, MAXG)).copy()
    ident = np.eye(128, dtype=np.float32).astype(ml_dtypes.bfloat16)

    in_maps = []
    for c in range(NCORES):
        seq_core = seq[c * BC:(c + 1) * BC].reshape(N)
        # device layout: seq_dev[p, i] = seq_core[i*128 + p]
        seq_dev = np.ascontiguousarray(seq_core.reshape(NSUB, 128).T)
        in_maps.append({
            "seq": seq_dev,
            "ctab": ctab,
            "gtab": gtab,
            "posrot": posrot,
            "giota": giota,
            "iota8": iota8,
            "ident": ident,
        })
    return in_maps


def postprocess(results):
    """Un-permute per-core outputs and concatenate to [B, L, D]."""
    outs = []
    for c in range(NCORES):
        o = results[c]["out"]  # [128, NSUB, D]
        outs.append(np.ascontiguousarray(o.transpose(1, 0, 2)).reshape(BC, L, D))
    return np.concatenate(outs, axis=0)


def kernel(sequence, token_table, genre_table, pos_table, token_genre_ids,
           genre_counts):
    nc = _get_nc()
    in_maps = prep_host_inputs(sequence, token_table, genre_table, pos_table,
                               token_genre_ids, genre_counts)
    res = run_bass_kernel_spmd(nc, in_maps, core_ids=list(range(NCORES)))
    return postprocess(res.results)


# revision 15
# speedup vs baseline: 1.0571x; 1.0218x over previous
"""Trainium2 Bass kernel for nn_BERTEmbedding_65274912964883.

out[b, l, :] = token_table[seq[b, l]]
             + mean_{g in genres(seq[b, l])} genre_table[g]
             + pos_table[l]

Strategy (8 NeuronCores, SPMD, no collectives):
  - Data-parallel over batch: 256 sequences -> 32 per core (6400 tokens/core).
  - One combined f32 table [VOCAB, 144] replicated per core:
    cols 0..127 token embedding, 128..135 genre ids, 136 count.
  - Per 128-token subtile (token t on partition t % 128): ONE indirect-DMA
    gather of 576B rows. The SWDGE descriptor emission (~9.6ns/row on the
    GpSimd Q7) paces the kernel; all other engines are kept beneath it.
  - genre mean = (one-hot histogram over 21 genres) @ genre_table:
    padded genre slots are remapped out of range (gid + 32*(1-mask));
    the one-hot cube is written in (j, g, s) layout so the s-reduction
    reads contiguously; normalization (x 1/count) is one small DVE op that
    also downcasts to bf16 for the PE; per-subtile PE transposes (base
    partition 0) feed K=21 bf16 matmuls; PSUM->SBUF histogram copies ride
    the otherwise-idle Scalar engine.
  - token + genre + positional adds are group-batched ([128, 512] PSUM
    banks), all f32.
  - positional rows come from a host-prebuilt rotated table (28 rotations,
    f32) -- a single startup DMA, no wrap handling.
  - Macro tiles are tapered [12, 12, 12, 12, 2] so the serial compute tail
    after the last gather is short.
  - Device writes output partition-major [128, N/128, D] f32; host
    un-permutes.
"""

import numpy as np
import ml_dtypes

import concourse.bacc as bacc
import concourse.mybir as mybir
import concourse.tile as tile
from concourse.bass import IndirectOffsetOnAxis
from concourse.bass_utils import run_bass_kernel_spmd

VOCAB = 100000
D = 128
G = 21          # genre ids are in [0, 20]
MAXG = 8
CW = 144        # combined-table row: 128 emb + 8 gid + 1 cnt + 7 pad (f32)
B, L = 256, 200
NCORES = 8
BC = B // NCORES          # sequences per core
N = BC * L                # tokens per core (6400)
SUB = 128                 # tokens per subtile (partition dim)
NSUB = N // SUB           # 50
MACROS = [12, 12, 12, 6, 4, 2, 1, 1]   # subtiles per macro tile (sum = NSUB)
NROT = 25                 # distinct values of (128*i) % 200
NROTX = 28                # extended with 3 duplicates so groups never wrap

F32 = mybir.dt.float32
BF16 = mybir.dt.bfloat16
I32 = mybir.dt.int32

assert sum(MACROS) == NSUB


def emit_core_kernel(tc, seq, ctab, gtab, posrot, giota, iota8, ident, out):
    """Emit the per-core kernel into TileContext `tc`.

    seq    : DRAM [128, NSUB] int32, seq[p, i] = token id of token i*128+p
    ctab   : DRAM [VOCAB, CW] f32 combined table
    gtab   : DRAM [G, D] bf16
    posrot : DRAM [128, NROTX*D] f32, posrot[p, r*D+d] = pos[(128r+p)%200, d]
    giota  : DRAM [128, G] f32, each row = 0..G-1
    iota8  : DRAM [128, MAXG] f32, each row = 0..MAXG-1
    ident  : DRAM [128, 128] bf16 identity
    out    : DRAM [128, NSUB, D] f32, out[p, i, :] = embedding of token i*128+p
    """
    nc = tc.nc
    add = mybir.AluOpType.add
    mult = mybir.AluOpType.mult

    with (
        tc.tile_pool(name="const", bufs=1) as cpool,
        tc.tile_pool(name="work", bufs=2) as wpool,
        tc.tile_pool(name="psum", bufs=2, space="PSUM") as ppool,
    ):
        # --- one-time loads; seq first (gathers depend only on it) ---
        seq_sb = cpool.tile([128, NSUB], I32)
        nc.sync.dma_start(out=seq_sb[:], in_=seq)
        gtab_sb = cpool.tile([G, D], BF16)
        nc.sync.dma_start(out=gtab_sb[:], in_=gtab)
        giota_sb = cpool.tile([128, G], F32)
        nc.sync.dma_start(out=giota_sb[:], in_=giota)
        iota8_sb = cpool.tile([128, MAXG], F32)
        nc.sync.dma_start(out=iota8_sb[:], in_=iota8)
        ident_sb = cpool.tile([128, 128], BF16)
        nc.sync.dma_start(out=ident_sb[:], in_=ident)
        posrot_sb = cpool.tile([128, NROTX * D], F32)
        nc.sync.dma_start(out=posrot_sb[:], in_=posrot)

        # --- main loop over macro tiles ---
        i0 = 0  # global subtile index of the macro's first subtile
        for ksub in MACROS:
            # gather combined rows, one indirect DMA per 128-token subtile
            cg_sb = wpool.tile([128, ksub * CW], F32, tag="cg", bufs=4)
            for j in range(ksub):
                nc.gpsimd.indirect_dma_start(
                    out=cg_sb[:, j * CW:(j + 1) * CW],
                    out_offset=None,
                    in_=ctab,
                    in_offset=IndirectOffsetOnAxis(
                        ap=seq_sb[:, i0 + j:i0 + j + 1], axis=0
                    ),
                )
            cg3 = cg_sb[:].rearrange("p (j c) -> p j c", c=CW)
            gid = cg3[:, :, D:D + MAXG]                # [128, ksub, MAXG]
            cnt = cg3[:, :, D + MAXG:D + MAXG + 1]     # [128, ksub, 1]

            # rec[p, j] = 1 / count
            rec_sb = wpool.tile([128, ksub], F32, tag="rec")
            nc.vector.reciprocal(rec_sb[:], cg3[:, :, D + MAXG])

            # mask[p, j, s] = (s < count[p, j])
            mask_sb = wpool.tile([128, ksub * MAXG], F32, tag="mask")
            m3 = mask_sb[:].rearrange("p (j s) -> p j s", s=MAXG)
            nc.vector.tensor_tensor(
                out=m3,
                in0=iota8_sb[:].unsqueeze(1).broadcast_to([128, ksub, MAXG]),
                in1=cnt.broadcast_to([128, ksub, MAXG]),
                op=mybir.AluOpType.is_lt,
            )
            # shift = 32 * (1 - mask); gidm = gid + shift
            # (padded slots land at >= 32 and never match any genre column)
            shift_sb = wpool.tile([128, ksub * MAXG], F32, tag="shift")
            nc.vector.tensor_scalar(
                out=shift_sb[:], in0=mask_sb[:],
                scalar1=-32.0, scalar2=32.0,
                op0=mult, op1=add,
            )
            gidm_sb = wpool.tile([128, ksub * MAXG], F32, tag="gidm")
            nc.vector.tensor_tensor(
                out=gidm_sb[:].rearrange("p (j s) -> p j s", s=MAXG),
                in0=gid,
                in1=shift_sb[:].rearrange("p (j s) -> p j s", s=MAXG),
                op=add,
            )

            # eq[p, j, s, g] = (gidm[p, j, s] == g)   (contiguous write)
            eq_sb = wpool.tile([128, ksub * MAXG * G], F32, tag="eq")
            e4 = eq_sb[:].rearrange("p (j s g) -> p j s g", s=MAXG, g=G)
            nc.vector.tensor_tensor(
                out=e4,
                in0=gidm_sb[:].rearrange("p (j s) -> p j s", s=MAXG)
                    .unsqueeze(3).broadcast_to([128, ksub, MAXG, G]),
                in1=giota_sb[:].unsqueeze(1).unsqueeze(2).broadcast_to(
                    [128, ksub, MAXG, G]
                ),
                op=mybir.AluOpType.is_equal,
            )

            # hist_raw[p, j, g] = sum_s eq[p, j, s, g] -- log-tree of adds so
            # every read is a contiguous 21-element run (the strided
            # reduce_sum reads single elements at stride 21 and is ~1.8x
            # slower)
            t1_sb = wpool.tile([128, ksub * 4 * G], F32, tag="tree1")
            t14 = t1_sb[:].rearrange("p (j s g) -> p j s g", s=4, g=G)
            nc.vector.tensor_tensor(
                out=t14, in0=e4[:, :, 0:4, :], in1=e4[:, :, 4:8, :], op=add)
            t2_sb = wpool.tile([128, ksub * 2 * G], F32, tag="tree2")
            t24 = t2_sb[:].rearrange("p (j s g) -> p j s g", s=2, g=G)
            nc.vector.tensor_tensor(
                out=t24, in0=t14[:, :, 0:2, :], in1=t14[:, :, 2:4, :], op=add)
            hist_sb = wpool.tile([128, ksub * G], F32, tag="hist")
            nc.vector.tensor_tensor(
                out=hist_sb[:].rearrange("p (j g) -> p j g", g=G),
                in0=t24[:, :, 0, :], in1=t24[:, :, 1, :], op=add)
            # hist_norm = hist_raw / count   (bf16 for the PE)
            histn_sb = wpool.tile([128, ksub * G], BF16, tag="histn")
            nc.vector.tensor_tensor(
                out=histn_sb[:].rearrange("p (j g) -> p j g", g=G),
                in0=hist_sb[:].rearrange("p (j g) -> p j g", g=G),
                in1=rec_sb[:].unsqueeze(2).broadcast_to([128, ksub, G]),
                op=mult,
            )

            # per-subtile PE transpose of the histogram (base partition 0);
            # PSUM -> SBUF copies ride the otherwise-idle Scalar engine
            histT = []
            for j in range(ksub):
                hT_ps = ppool.tile([G, 128], BF16, tag="hT_ps", bufs=3)
                nc.tensor.transpose(
                    out=hT_ps[:],
                    in_=histn_sb[:, j * G:(j + 1) * G],
                    identity=ident_sb[:],
                )
                hT_sb = wpool.tile([G, 128], BF16, tag="hT_sb", bufs=3)
                nc.scalar.copy(out=hT_sb[:], in_=hT_ps[:])
                histT.append(hT_sb)

            out_sb = wpool.tile([128, ksub * D], F32, tag="outsb")
            for j0 in range(0, ksub, 4):
                ng = min(4, ksub - j0)
                gm_ps = ppool.tile([128, ng * D], F32, tag="gm_ps")
                for k in range(ng):
                    nc.tensor.matmul(
                        out=gm_ps[:, k * D:(k + 1) * D],
                        lhsT=histT[j0 + k][:],
                        rhs=gtab_sb[:],
                        start=True, stop=True,
                    )
                # out = tok + genre_mean + pos   (two group-batched f32 adds)
                oslice = out_sb[:, j0 * D:(j0 + ng) * D]
                nc.vector.tensor_tensor(
                    out=oslice,
                    in0=cg3[:, j0:j0 + ng, 0:D],
                    in1=gm_ps[:],
                    op=add,
                )
                r0 = (i0 + j0) % NROT
                nc.vector.tensor_tensor(
                    out=oslice,
                    in0=oslice,
                    in1=posrot_sb[:, r0 * D:(r0 + ng) * D],
                    op=add,
                )
                # store per group (spreads SDMA ring load, shortens the tail)
                nc.sync.dma_start(
                    out=out[:, i0 + j0:i0 + j0 + ng, :],
                    in_=out_sb[:, j0 * D:(j0 + ng) * D]
                        .rearrange("p (j d) -> p j d", d=D),
                )
            i0 += ksub


def build_nc():
    nc = bacc.Bacc("TRN2", target_bir_lowering=False, debug=False)
    seq = nc.dram_tensor("seq", [128, NSUB], I32, kind="ExternalInput").ap()
    ctab = nc.dram_tensor("ctab", [VOCAB, CW], F32, kind="ExternalInput").ap()
    gtab = nc.dram_tensor("gtab", [G, D], BF16, kind="ExternalInput").ap()
    posrot = nc.dram_tensor(
        "posrot", [128, NROTX * D], F32, kind="ExternalInput").ap()
    giota = nc.dram_tensor("giota", [128, G], F32, kind="ExternalInput").ap()
    iota8 = nc.dram_tensor("iota8", [128, MAXG], F32, kind="ExternalInput").ap()
    ident = nc.dram_tensor("ident", [128, 128], BF16, kind="ExternalInput").ap()
    out = nc.dram_tensor("out", [128, NSUB, D], F32, kind="ExternalOutput").ap()

    with tile.TileContext(nc) as tc:
        emit_core_kernel(tc, seq, ctab, gtab, posrot, giota, iota8, ident, out)
    nc.compile()
    return nc


_NC_CACHE = None


def _get_nc():
    global _NC_CACHE
    if _NC_CACHE is None:
        _NC_CACHE = build_nc()
    return _NC_CACHE


def make_ctab(token_table, token_genre_ids, genre_counts):
    ctab = np.zeros((VOCAB, CW), dtype=np.float32)
    ctab[:, 0:D] = np.asarray(token_table, dtype=np.float32)
    ctab[:, D:D + MAXG] = np.asarray(token_genre_ids, dtype=np.float32)
    ctab[:, D + MAXG] = np.asarray(genre_counts, dtype=np.float32)
    return ctab


def make_posrot(pos_table):
    pos = np.asarray(pos_table, dtype=np.float32)
    pr = np.zeros((128, NROTX * D), dtype=np.float32)
    p = np.arange(128)
    for r in range(NROTX):
        pr[:, r * D:(r + 1) * D] = pos[(128 * r + p) % L, :]
    return pr


def prep_host_inputs(sequence, token_table, genre_table, pos_table,
                     token_genre_ids, genre_counts):
    """Host-side sharding / layout prep. Returns in_maps for the 8 cores."""
    seq = np.ascontiguousarray(np.asarray(sequence).astype(np.int32)).reshape(B, L)
    ctab = make_ctab(token_table, token_genre_ids, genre_counts)
    gtab = np.asarray(genre_table, dtype=np.float32).astype(ml_dtypes.bfloat16)
    posrot = make_posrot(pos_table)

    giota = np.broadcast_to(np.arange(G, dtype=np.float32), (128, G)).copy()
    iota8 = np.broadcast_to(
        np.arange(MAXG, dtype=np.float32), (128, MAXG)).copy()
    ident = np.eye(128, dtype=np.float32).astype(ml_dtypes.bfloat16)

    in_maps = []
    for c in range(NCORES):
        seq_core = seq[c * BC:(c + 1) * BC].reshape(N)
        # device layout: seq_dev[p, i] = seq_core[i*128 + p]
        seq_dev = np.ascontiguousarray(seq_core.reshape(NSUB, 128).T)
        in_maps.append({
            "seq": seq_dev,
            "ctab": ctab,
            "gtab": gtab,
            "posrot": posrot,
            "giota": giota,
            "iota8": iota8,
            "ident": ident,
        })
    return in_maps


def postprocess(results):
    """Un-permute per-core outputs and concatenate to [B, L, D]."""
    outs = []
    for c in range(NCORES):
        o = results[c]["out"]  # [128, NSUB, D]
        outs.append(np.ascontiguousarray(o.transpose(1, 0, 2)).reshape(BC, L, D))
    return np.concatenate(outs, axis=0)


def kernel(sequence, token_table, genre_table, pos_table, token_genre_ids,
           genre_counts):
    nc = _get_nc()
    in_maps = prep_host_inputs(sequence, token_table, genre_table, pos_table,
                               token_genre_ids, genre_counts)
    res = run_bass_kernel_spmd(nc, in_maps, core_ids=list(range(NCORES)))
    return postprocess(res.results)


# revision 16
# speedup vs baseline: 1.0709x; 1.0131x over previous
"""Trainium2 Bass kernel for nn_BERTEmbedding_65274912964883.

out[b, l, :] = token_table[seq[b, l]]
             + mean_{g in genres(seq[b, l])} genre_table[g]
             + pos_table[l]

Strategy (8 NeuronCores, SPMD, no collectives):
  - Data-parallel over batch: 256 sequences -> 32 per core (6400 tokens/core).
  - One combined f32 table [VOCAB, 144] replicated per core:
    cols 0..127 token embedding, 128..135 genre ids, 136 count.
  - Per 128-token subtile (token t on partition t % 128): ONE indirect-DMA
    gather of 576B rows. The SWDGE descriptor emission (~9.6ns/row on the
    GpSimd Q7) paces the kernel; all other engines are kept beneath it.
  - genre mean = (one-hot histogram over 21 genres) @ genre_table:
    padded genre slots are remapped out of range (gid + 32*(1-mask));
    the one-hot cube is written in (j, g, s) layout so the s-reduction
    reads contiguously; normalization (x 1/count) is one small DVE op that
    also downcasts to bf16 for the PE; per-subtile PE transposes (base
    partition 0) feed K=21 bf16 matmuls; PSUM->SBUF histogram copies ride
    the otherwise-idle Scalar engine.
  - token + genre + positional adds are group-batched ([128, 512] PSUM
    banks), all f32.
  - positional rows come from a host-prebuilt rotated table (28 rotations,
    f32) -- a single startup DMA, no wrap handling.
  - Macro tiles are tapered [12, 12, 12, 12, 2] so the serial compute tail
    after the last gather is short.
  - Device writes output partition-major [128, N/128, D] f32; host
    un-permutes.
"""

import numpy as np
import ml_dtypes

import concourse.bacc as bacc
import concourse.mybir as mybir
import concourse.tile as tile
from concourse.bass import IndirectOffsetOnAxis
from concourse.bass_utils import run_bass_kernel_spmd

VOCAB = 100000
D = 128
G = 21          # genre ids are in [0, 20]
MAXG = 8
CW = 144        # combined-table row: 128 emb + 8 gid + 1 cnt + 7 pad (bf16)
B, L = 256, 200
NCORES = 8
BC = B // NCORES          # sequences per core
N = BC * L                # tokens per core (6400)
SUB = 128                 # tokens per subtile (partition dim)
NSUB = N // SUB           # 50
MACROS = [12, 12, 12, 6, 4, 2, 1, 1]   # subtiles per macro tile (sum = NSUB)
NROT = 25                 # distinct values of (128*i) % 200
NROTX = 28                # extended with 3 duplicates so groups never wrap

F32 = mybir.dt.float32
BF16 = mybir.dt.bfloat16
I32 = mybir.dt.int32

assert sum(MACROS) == NSUB


def emit_core_kernel(tc, seq, ctab, gtab, posrot, giota, iota8, ident, out):
    """Emit the per-core kernel into TileContext `tc`.

    seq    : DRAM [128, NSUB] int32, seq[p, i] = token id of token i*128+p
    ctab   : DRAM [VOCAB, CW] bf16 combined table
    gtab   : DRAM [G, D] bf16
    posrot : DRAM [128, NROTX*D] bf16
    giota  : DRAM [128, G] bf16, each row = 0..G-1
    iota8  : DRAM [128, MAXG] bf16, each row = 0..MAXG-1
    ident  : DRAM [128, 128] bf16 identity
    out    : DRAM [128, NSUB, D] f32, out[p, i, :] = embedding of token i*128+p
    """
    nc = tc.nc
    add = mybir.AluOpType.add
    mult = mybir.AluOpType.mult

    with (
        tc.tile_pool(name="const", bufs=1) as cpool,
        tc.tile_pool(name="work", bufs=2) as wpool,
        tc.tile_pool(name="psum", bufs=2, space="PSUM") as ppool,
    ):
        # --- one-time loads; seq first (gathers depend only on it) ---
        seq_sb = cpool.tile([128, NSUB], I32)
        nc.sync.dma_start(out=seq_sb[:], in_=seq)
        gtab_sb = cpool.tile([G, D], BF16)
        nc.sync.dma_start(out=gtab_sb[:], in_=gtab)
        giota_sb = cpool.tile([128, G], BF16)
        nc.sync.dma_start(out=giota_sb[:], in_=giota)
        iota8_sb = cpool.tile([128, MAXG], BF16)
        nc.sync.dma_start(out=iota8_sb[:], in_=iota8)
        ident_sb = cpool.tile([128, 128], BF16)
        nc.sync.dma_start(out=ident_sb[:], in_=ident)
        posrot_sb = cpool.tile([128, NROTX * D], BF16)
        nc.sync.dma_start(out=posrot_sb[:], in_=posrot)

        # --- main loop over macro tiles ---
        i0 = 0  # global subtile index of the macro's first subtile
        for ksub in MACROS:
            # gather combined rows, one indirect DMA per 128-token subtile
            cg_sb = wpool.tile([128, ksub * CW], BF16, tag="cg", bufs=4)
            for j in range(ksub):
                nc.gpsimd.indirect_dma_start(
                    out=cg_sb[:, j * CW:(j + 1) * CW],
                    out_offset=None,
                    in_=ctab,
                    in_offset=IndirectOffsetOnAxis(
                        ap=seq_sb[:, i0 + j:i0 + j + 1], axis=0
                    ),
                )
            cg3 = cg_sb[:].rearrange("p (j c) -> p j c", c=CW)
            gid = cg3[:, :, D:D + MAXG]                # [128, ksub, MAXG]
            cnt = cg3[:, :, D + MAXG:D + MAXG + 1]     # [128, ksub, 1]

            # rec[p, j] = 1 / count
            rec_sb = wpool.tile([128, ksub], F32, tag="rec")
            nc.vector.reciprocal(rec_sb[:], cg3[:, :, D + MAXG])

            # mask[p, j, s] = (s < count[p, j])
            mask_sb = wpool.tile([128, ksub * MAXG], BF16, tag="mask")
            m3 = mask_sb[:].rearrange("p (j s) -> p j s", s=MAXG)
            nc.vector.tensor_tensor(
                out=m3,
                in0=iota8_sb[:].unsqueeze(1).broadcast_to([128, ksub, MAXG]),
                in1=cnt.broadcast_to([128, ksub, MAXG]),
                op=mybir.AluOpType.is_lt,
            )
            # shift = 32 * (1 - mask); gidm = gid + shift
            # (padded slots land at >= 32 and never match any genre column)
            shift_sb = wpool.tile([128, ksub * MAXG], BF16, tag="shift")
            nc.vector.tensor_scalar(
                out=shift_sb[:], in0=mask_sb[:],
                scalar1=-32.0, scalar2=32.0,
                op0=mult, op1=add,
            )
            gidm_sb = wpool.tile([128, ksub * MAXG], BF16, tag="gidm")
            nc.vector.tensor_tensor(
                out=gidm_sb[:].rearrange("p (j s) -> p j s", s=MAXG),
                in0=gid,
                in1=shift_sb[:].rearrange("p (j s) -> p j s", s=MAXG),
                op=add,
            )

            # eq[p, j, s, g] = (gidm[p, j, s] == g)   (contiguous write)
            eq_sb = wpool.tile([128, ksub * MAXG * G], BF16, tag="eq")
            e4 = eq_sb[:].rearrange("p (j s g) -> p j s g", s=MAXG, g=G)
            nc.vector.tensor_tensor(
                out=e4,
                in0=gidm_sb[:].rearrange("p (j s) -> p j s", s=MAXG)
                    .unsqueeze(3).broadcast_to([128, ksub, MAXG, G]),
                in1=giota_sb[:].unsqueeze(1).unsqueeze(2).broadcast_to(
                    [128, ksub, MAXG, G]
                ),
                op=mybir.AluOpType.is_equal,
            )

            # hist_raw[p, j, g] = sum_s eq[p, j, s, g] -- log-tree of adds so
            # every read is a contiguous 21-element run (the strided
            # reduce_sum reads single elements at stride 21 and is ~1.8x
            # slower)
            t1_sb = wpool.tile([128, ksub * 4 * G], BF16, tag="tree1")
            t14 = t1_sb[:].rearrange("p (j s g) -> p j s g", s=4, g=G)
            nc.vector.tensor_tensor(
                out=t14, in0=e4[:, :, 0:4, :], in1=e4[:, :, 4:8, :], op=add)
            t2_sb = wpool.tile([128, ksub * 2 * G], BF16, tag="tree2")
            t24 = t2_sb[:].rearrange("p (j s g) -> p j s g", s=2, g=G)
            nc.vector.tensor_tensor(
                out=t24, in0=t14[:, :, 0:2, :], in1=t14[:, :, 2:4, :], op=add)
            hist_sb = wpool.tile([128, ksub * G], BF16, tag="hist")
            nc.vector.tensor_tensor(
                out=hist_sb[:].rearrange("p (j g) -> p j g", g=G),
                in0=t24[:, :, 0, :], in1=t24[:, :, 1, :], op=add)
            # hist_norm = hist_raw / count   (bf16 for the PE)
            histn_sb = wpool.tile([128, ksub * G], BF16, tag="histn")
            nc.vector.tensor_tensor(
                out=histn_sb[:].rearrange("p (j g) -> p j g", g=G),
                in0=hist_sb[:].rearrange("p (j g) -> p j g", g=G),
                in1=rec_sb[:].unsqueeze(2).broadcast_to([128, ksub, G]),
                op=mult,
            )

            # per-subtile PE transpose of the histogram (base partition 0);
            # PSUM -> SBUF copies ride the otherwise-idle Scalar engine
            histT = []
            for j in range(ksub):
                hT_ps = ppool.tile([G, 128], BF16, tag="hT_ps", bufs=3)
                nc.tensor.transpose(
                    out=hT_ps[:],
                    in_=histn_sb[:, j * G:(j + 1) * G],
                    identity=ident_sb[:],
                )
                hT_sb = wpool.tile([G, 128], BF16, tag="hT_sb", bufs=3)
                nc.scalar.copy(out=hT_sb[:], in_=hT_ps[:])
                histT.append(hT_sb)

            out_sb = wpool.tile([128, ksub * D], F32, tag="outsb")
            for j0 in range(0, ksub, 4):
                ng = min(4, ksub - j0)
                gm_ps = ppool.tile([128, ng * D], F32, tag="gm_ps")
                for k in range(ng):
                    nc.tensor.matmul(
                        out=gm_ps[:, k * D:(k + 1) * D],
                        lhsT=histT[j0 + k][:],
                        rhs=gtab_sb[:],
                        start=True, stop=True,
                    )
                # out = tok + genre_mean + pos   (two group-batched f32 adds)
                oslice = out_sb[:, j0 * D:(j0 + ng) * D]
                nc.vector.tensor_tensor(
                    out=oslice,
                    in0=cg3[:, j0:j0 + ng, 0:D],
                    in1=gm_ps[:],
                    op=add,
                )
                r0 = (i0 + j0) % NROT
                nc.vector.tensor_tensor(
                    out=oslice,
                    in0=oslice,
                    in1=posrot_sb[:, r0 * D:(r0 + ng) * D],
                    op=add,
                )
                # store per group (spreads SDMA ring load, shortens the tail)
                nc.sync.dma_start(
                    out=out[:, i0 + j0:i0 + j0 + ng, :],
                    in_=out_sb[:, j0 * D:(j0 + ng) * D]
                        .rearrange("p (j d) -> p j d", d=D),
                )
            i0 += ksub


def build_nc():
    nc = bacc.Bacc("TRN2", target_bir_lowering=False, debug=False)
    seq = nc.dram_tensor("seq", [128, NSUB], I32, kind="ExternalInput").ap()
    ctab = nc.dram_tensor("ctab", [VOCAB, CW], BF16, kind="ExternalInput").ap()
    gtab = nc.dram_tensor("gtab", [G, D], BF16, kind="ExternalInput").ap()
    posrot = nc.dram_tensor(
        "posrot", [128, NROTX * D], BF16, kind="ExternalInput").ap()
    giota = nc.dram_tensor("giota", [128, G], BF16, kind="ExternalInput").ap()
    iota8 = nc.dram_tensor("iota8", [128, MAXG], BF16, kind="ExternalInput").ap()
    ident = nc.dram_tensor("ident", [128, 128], BF16, kind="ExternalInput").ap()
    out = nc.dram_tensor("out", [128, NSUB, D], F32, kind="ExternalOutput").ap()

    with tile.TileContext(nc) as tc:
        emit_core_kernel(tc, seq, ctab, gtab, posrot, giota, iota8, ident, out)
    nc.compile()
    return nc


_NC_CACHE = None


def _get_nc():
    global _NC_CACHE
    if _NC_CACHE is None:
        _NC_CACHE = build_nc()
    return _NC_CACHE


def make_ctab(token_table, token_genre_ids, genre_counts):
    ctab = np.zeros((VOCAB, CW), dtype=ml_dtypes.bfloat16)
    ctab[:, 0:D] = np.asarray(token_table, dtype=np.float32).astype(
        ml_dtypes.bfloat16)
    ctab[:, D:D + MAXG] = np.asarray(
        token_genre_ids, dtype=np.float32).astype(ml_dtypes.bfloat16)
    ctab[:, D + MAXG] = np.asarray(
        genre_counts, dtype=np.float32).astype(ml_dtypes.bfloat16)
    return ctab


def make_posrot(pos_table):
    pos = np.asarray(pos_table, dtype=np.float32)
    pr = np.zeros((128, NROTX * D), dtype=np.float32)
    p = np.arange(128)
    for r in range(NROTX):
        pr[:, r * D:(r + 1) * D] = pos[(128 * r + p) % L, :]
    return pr.astype(ml_dtypes.bfloat16)


def prep_host_inputs(sequence, token_table, genre_table, pos_table,
                     token_genre_ids, genre_counts):
    """Host-side sharding / layout prep. Returns in_maps for the 8 cores."""
    seq = np.ascontiguousarray(np.asarray(sequence).astype(np.int32)).reshape(B, L)
    ctab = make_ctab(token_table, token_genre_ids, genre_counts)
    gtab = np.asarray(genre_table, dtype=np.float32).astype(ml_dtypes.bfloat16)
    posrot = make_posrot(pos_table)

    giota = np.broadcast_to(
        np.arange(G, dtype=np.float32), (128, G)).astype(ml_dtypes.bfloat16)
    iota8 = np.broadcast_to(
        np.arange(MAXG, dtype=np.float32), (128, MAXG)).astype(
        ml_dtypes.bfloat16)
    ident = np.eye(128, dtype=np.float32).astype(ml_dtypes.bfloat16)

    in_maps = []
    for c in range(NCORES):
        seq_core = seq[c * BC:(c + 1) * BC].reshape(N)
        # device layout: seq_dev[p, i] = seq_core[i*128 + p]
        seq_dev = np.ascontiguousarray(seq_core.reshape(NSUB, 128).T)
        in_maps.append({
            "seq": seq_dev,
            "ctab": ctab,
            "gtab": gtab,
            "posrot": posrot,
            "giota": giota,
            "iota8": iota8,
            "ident": ident,
        })
    return in_maps


def postprocess(results):
    """Un-permute per-core outputs and concatenate to [B, L, D]."""
    outs = []
    for c in range(NCORES):
        o = results[c]["out"]  # [128, NSUB, D]
        outs.append(np.ascontiguousarray(o.transpose(1, 0, 2)).reshape(BC, L, D))
    return np.concatenate(outs, axis=0)


def kernel(sequence, token_table, genre_table, pos_table, token_genre_ids,
           genre_counts):
    nc = _get_nc()
    in_maps = prep_host_inputs(sequence, token_table, genre_table, pos_table,
                               token_genre_ids, genre_counts)
    res = run_bass_kernel_spmd(nc, in_maps, core_ids=list(range(NCORES)))
    return postprocess(res.results)


# revision 17
# speedup vs baseline: 1.0965x; 1.0239x over previous
"""Trainium2 Bass kernel for nn_BERTEmbedding_65274912964883.

out[b, l, :] = token_table[seq[b, l]]
             + mean_{g in genres(seq[b, l])} genre_table[g]
             + pos_table[l]

Strategy (8 NeuronCores, SPMD, no collectives):
  - Data-parallel over batch: 256 sequences -> 32 per core (6400 tokens/core).
  - One combined f32 table [VOCAB, 144] replicated per core:
    cols 0..127 token embedding, 128..135 genre ids, 136 count.
  - Per 128-token subtile (token t on partition t % 128): ONE indirect-DMA
    gather of 576B rows. The SWDGE descriptor emission (~9.6ns/row on the
    GpSimd Q7) paces the kernel; all other engines are kept beneath it.
  - genre mean = (one-hot histogram over 21 genres) @ genre_table:
    padded genre slots are remapped out of range (gid + 32*(1-mask));
    the one-hot cube is written in (j, g, s) layout so the s-reduction
    reads contiguously; normalization (x 1/count) is one small DVE op that
    also downcasts to bf16 for the PE; per-subtile PE transposes (base
    partition 0) feed K=21 bf16 matmuls; PSUM->SBUF histogram copies ride
    the otherwise-idle Scalar engine.
  - token + genre + positional adds are group-batched ([128, 512] PSUM
    banks), all f32.
  - positional rows come from a host-prebuilt rotated table (28 rotations,
    f32) -- a single startup DMA, no wrap handling.
  - Macro tiles are tapered [12, 12, 12, 12, 2] so the serial compute tail
    after the last gather is short.
  - Device writes output partition-major [128, N/128, D] f32; host
    un-permutes.
"""

import numpy as np
import ml_dtypes

import concourse.bacc as bacc
import concourse.mybir as mybir
import concourse.tile as tile
from concourse.bass import IndirectOffsetOnAxis
from concourse.bass_utils import run_bass_kernel_spmd

VOCAB = 100000
D = 128
G = 21          # genre ids are in [0, 20]
MAXG = 8
CW = 144        # combined-table row: 128 emb + 8 gid + 1 cnt + 7 pad (bf16)
B, L = 256, 200
NCORES = 8
BC = B // NCORES          # sequences per core
N = BC * L                # tokens per core (6400)
SUB = 128                 # tokens per subtile (partition dim)
NSUB = N // SUB           # 50
MACROS = [12, 12, 12, 6, 4, 2, 1, 1]   # subtiles per macro tile (sum = NSUB)
NROT = 25                 # distinct values of (128*i) % 200
NROTX = 28                # extended with 3 duplicates so groups never wrap

F32 = mybir.dt.float32
BF16 = mybir.dt.bfloat16
I32 = mybir.dt.int32

assert sum(MACROS) == NSUB


def emit_core_kernel(tc, seq, ctab, gtab, posrot, giota, iota8, ident, out):
    """Emit the per-core kernel into TileContext `tc`.

    seq    : DRAM [128, NSUB] int32, seq[p, i] = token id of token i*128+p
    ctab   : DRAM [VOCAB, CW] bf16 combined table
    gtab   : DRAM [G, D] bf16
    posrot : DRAM [128, NROTX*D] bf16
    giota  : DRAM [128, G] bf16, each row = 0..G-1
    iota8  : DRAM [128, MAXG] bf16, each row = 0..MAXG-1
    ident  : DRAM [128, 128] bf16 identity
    out    : DRAM [128, NSUB, D] f32, out[p, i, :] = embedding of token i*128+p
    """
    nc = tc.nc
    add = mybir.AluOpType.add
    mult = mybir.AluOpType.mult

    with (
        tc.tile_pool(name="const", bufs=1) as cpool,
        tc.tile_pool(name="work", bufs=2) as wpool,
        tc.tile_pool(name="psum", bufs=2, space="PSUM") as ppool,
    ):
        # --- one-time loads; seq first (gathers depend only on it) ---
        seq_sb = cpool.tile([128, NSUB], I32)
        nc.sync.dma_start(out=seq_sb[:], in_=seq)
        gtab_sb = cpool.tile([G, D], BF16)
        nc.sync.dma_start(out=gtab_sb[:], in_=gtab)
        giota_sb = cpool.tile([128, G], BF16)
        nc.sync.dma_start(out=giota_sb[:], in_=giota)
        iota8_sb = cpool.tile([128, MAXG], BF16)
        nc.sync.dma_start(out=iota8_sb[:], in_=iota8)
        ident_sb = cpool.tile([128, 128], BF16)
        nc.sync.dma_start(out=ident_sb[:], in_=ident)
        posrot_sb = cpool.tile([128, NROTX * D], BF16)
        nc.sync.dma_start(out=posrot_sb[:], in_=posrot)

        # --- main loop over macro tiles ---
        i0 = 0  # global subtile index of the macro's first subtile
        for ksub in MACROS:
            # gather combined rows, one indirect DMA per 128-token subtile
            cg_sb = wpool.tile([128, ksub * CW], BF16, tag="cg", bufs=8)
            for j in range(ksub):
                nc.gpsimd.indirect_dma_start(
                    out=cg_sb[:, j * CW:(j + 1) * CW],
                    out_offset=None,
                    in_=ctab,
                    in_offset=IndirectOffsetOnAxis(
                        ap=seq_sb[:, i0 + j:i0 + j + 1], axis=0
                    ),
                )
            cg3 = cg_sb[:].rearrange("p (j c) -> p j c", c=CW)
            gid = cg3[:, :, D:D + MAXG]                # [128, ksub, MAXG]
            cnt = cg3[:, :, D + MAXG:D + MAXG + 1]     # [128, ksub, 1]

            # rec[p, j] = 1 / count
            rec_sb = wpool.tile([128, ksub], F32, tag="rec")
            nc.vector.reciprocal(rec_sb[:], cg3[:, :, D + MAXG])

            # mask[p, j, s] = (s < count[p, j])
            mask_sb = wpool.tile([128, ksub * MAXG], BF16, tag="mask")
            m3 = mask_sb[:].rearrange("p (j s) -> p j s", s=MAXG)
            nc.vector.tensor_tensor(
                out=m3,
                in0=iota8_sb[:].unsqueeze(1).broadcast_to([128, ksub, MAXG]),
                in1=cnt.broadcast_to([128, ksub, MAXG]),
                op=mybir.AluOpType.is_lt,
            )
            # shift = 32 * (1 - mask); gidm = gid + shift
            # (padded slots land at >= 32 and never match any genre column)
            shift_sb = wpool.tile([128, ksub * MAXG], BF16, tag="shift")
            nc.vector.tensor_scalar(
                out=shift_sb[:], in0=mask_sb[:],
                scalar1=-32.0, scalar2=32.0,
                op0=mult, op1=add,
            )
            gidm_sb = wpool.tile([128, ksub * MAXG], BF16, tag="gidm")
            nc.vector.tensor_tensor(
                out=gidm_sb[:].rearrange("p (j s) -> p j s", s=MAXG),
                in0=gid,
                in1=shift_sb[:].rearrange("p (j s) -> p j s", s=MAXG),
                op=add,
            )

            # eq[p, j, s, g] = (gidm[p, j, s] == g)   (contiguous write)
            eq_sb = wpool.tile([128, ksub * MAXG * G], BF16, tag="eq")
            e4 = eq_sb[:].rearrange("p (j s g) -> p j s g", s=MAXG, g=G)
            nc.vector.tensor_tensor(
                out=e4,
                in0=gidm_sb[:].rearrange("p (j s) -> p j s", s=MAXG)
                    .unsqueeze(3).broadcast_to([128, ksub, MAXG, G]),
                in1=giota_sb[:].unsqueeze(1).unsqueeze(2).broadcast_to(
                    [128, ksub, MAXG, G]
                ),
                op=mybir.AluOpType.is_equal,
            )

            # hist_raw[p, j, g] = sum_s eq[p, j, s, g] -- log-tree of adds so
            # every read is a contiguous 21-element run (the strided
            # reduce_sum reads single elements at stride 21 and is ~1.8x
            # slower)
            t1_sb = wpool.tile([128, ksub * 4 * G], BF16, tag="tree1")
            t14 = t1_sb[:].rearrange("p (j s g) -> p j s g", s=4, g=G)
            nc.vector.tensor_tensor(
                out=t14, in0=e4[:, :, 0:4, :], in1=e4[:, :, 4:8, :], op=add)
            t2_sb = wpool.tile([128, ksub * 2 * G], BF16, tag="tree2")
            t24 = t2_sb[:].rearrange("p (j s g) -> p j s g", s=2, g=G)
            nc.vector.tensor_tensor(
                out=t24, in0=t14[:, :, 0:2, :], in1=t14[:, :, 2:4, :], op=add)
            hist_sb = wpool.tile([128, ksub * G], BF16, tag="hist")
            nc.vector.tensor_tensor(
                out=hist_sb[:].rearrange("p (j g) -> p j g", g=G),
                in0=t24[:, :, 0, :], in1=t24[:, :, 1, :], op=add)
            # hist_norm = hist_raw / count   (bf16 for the PE)
            histn_sb = wpool.tile([128, ksub * G], BF16, tag="histn")
            nc.vector.tensor_tensor(
                out=histn_sb[:].rearrange("p (j g) -> p j g", g=G),
                in0=hist_sb[:].rearrange("p (j g) -> p j g", g=G),
                in1=rec_sb[:].unsqueeze(2).broadcast_to([128, ksub, G]),
                op=mult,
            )

            # per-subtile PE transpose of the histogram (base partition 0);
            # PSUM -> SBUF copies ride the otherwise-idle Scalar engine
            histT = []
            for j in range(ksub):
                hT_ps = ppool.tile([G, 128], BF16, tag="hT_ps", bufs=3)
                nc.tensor.transpose(
                    out=hT_ps[:],
                    in_=histn_sb[:, j * G:(j + 1) * G],
                    identity=ident_sb[:],
                )
                hT_sb = wpool.tile([G, 128], BF16, tag="hT_sb", bufs=3)
                nc.scalar.copy(out=hT_sb[:], in_=hT_ps[:])
                histT.append(hT_sb)

            out_sb = wpool.tile([128, ksub * D], F32, tag="outsb", bufs=3)
            for j0 in range(0, ksub, 4):
                ng = min(4, ksub - j0)
                gm_ps = ppool.tile([128, ng * D], F32, tag="gm_ps", bufs=3)
                for k in range(ng):
                    nc.tensor.matmul(
                        out=gm_ps[:, k * D:(k + 1) * D],
                        lhsT=histT[j0 + k][:],
                        rhs=gtab_sb[:],
                        start=True, stop=True,
                    )
                # out = tok + genre_mean + pos   (two group-batched f32 adds)
                oslice = out_sb[:, j0 * D:(j0 + ng) * D]
                nc.vector.tensor_tensor(
                    out=oslice,
                    in0=cg3[:, j0:j0 + ng, 0:D],
                    in1=gm_ps[:],
                    op=add,
                )
                r0 = (i0 + j0) % NROT
                nc.vector.tensor_tensor(
                    out=oslice,
                    in0=oslice,
                    in1=posrot_sb[:, r0 * D:(r0 + ng) * D],
                    op=add,
                )
                # store per group (spreads SDMA ring load, shortens the tail)
                nc.sync.dma_start(
                    out=out[:, i0 + j0:i0 + j0 + ng, :],
                    in_=out_sb[:, j0 * D:(j0 + ng) * D]
                        .rearrange("p (j d) -> p j d", d=D),
                )
            i0 += ksub


def build_nc():
    nc = bacc.Bacc("TRN2", target_bir_lowering=False, debug=False)
    seq = nc.dram_tensor("seq", [128, NSUB], I32, kind="ExternalInput").ap()
    ctab = nc.dram_tensor("ctab", [VOCAB, CW], BF16, kind="ExternalInput").ap()
    gtab = nc.dram_tensor("gtab", [G, D], BF16, kind="ExternalInput").ap()
    posrot = nc.dram_tensor(
        "posrot", [128, NROTX * D], BF16, kind="ExternalInput").ap()
    giota = nc.dram_tensor("giota", [128, G], BF16, kind="ExternalInput").ap()
    iota8 = nc.dram_tensor("iota8", [128, MAXG], BF16, kind="ExternalInput").ap()
    ident = nc.dram_tensor("ident", [128, 128], BF16, kind="ExternalInput").ap()
    out = nc.dram_tensor("out", [128, NSUB, D], F32, kind="ExternalOutput").ap()

    with tile.TileContext(nc) as tc:
        emit_core_kernel(tc, seq, ctab, gtab, posrot, giota, iota8, ident, out)
    nc.compile()
    return nc


_NC_CACHE = None


def _get_nc():
    global _NC_CACHE
    if _NC_CACHE is None:
        _NC_CACHE = build_nc()
    return _NC_CACHE


def make_ctab(token_table, token_genre_ids, genre_counts):
    ctab = np.zeros((VOCAB, CW), dtype=ml_dtypes.bfloat16)
    ctab[:, 0:D] = np.asarray(token_table, dtype=np.float32).astype(
        ml_dtypes.bfloat16)
    ctab[:, D:D + MAXG] = np.asarray(
        token_genre_ids, dtype=np.float32).astype(ml_dtypes.bfloat16)
    ctab[:, D + MAXG] = np.asarray(
        genre_counts, dtype=np.float32).astype(ml_dtypes.bfloat16)
    return ctab


def make_posrot(pos_table):
    pos = np.asarray(pos_table, dtype=np.float32)
    pr = np.zeros((128, NROTX * D), dtype=np.float32)
    p = np.arange(128)
    for r in range(NROTX):
        pr[:, r * D:(r + 1) * D] = pos[(128 * r + p) % L, :]
    return pr.astype(ml_dtypes.bfloat16)


def prep_host_inputs(sequence, token_table, genre_table, pos_table,
                     token_genre_ids, genre_counts):
    """Host-side sharding / layout prep. Returns in_maps for the 8 cores."""
    seq = np.ascontiguousarray(np.asarray(sequence).astype(np.int32)).reshape(B, L)
    ctab = make_ctab(token_table, token_genre_ids, genre_counts)
    gtab = np.asarray(genre_table, dtype=np.float32).astype(ml_dtypes.bfloat16)
    posrot = make_posrot(pos_table)

    giota = np.broadcast_to(
        np.arange(G, dtype=np.float32), (128, G)).astype(ml_dtypes.bfloat16)
    iota8 = np.broadcast_to(
        np.arange(MAXG, dtype=np.float32), (128, MAXG)).astype(
        ml_dtypes.bfloat16)
    ident = np.eye(128, dtype=np.float32).astype(ml_dtypes.bfloat16)

    in_maps = []
    for c in range(NCORES):
        seq_core = seq[c * BC:(c + 1) * BC].reshape(N)
        # device layout: seq_dev[p, i] = seq_core[i*128 + p]
        seq_dev = np.ascontiguousarray(seq_core.reshape(NSUB, 128).T)
        in_maps.append({
            "seq": seq_dev,
            "ctab": ctab,
            "gtab": gtab,
            "posrot": posrot,
            "giota": giota,
            "iota8": iota8,
            "ident": ident,
        })
    return in_maps


def postprocess(results):
    """Un-permute per-core outputs and concatenate to [B, L, D]."""
    outs = []
    for c in range(NCORES):
        o = results[c]["out"]  # [128, NSUB, D]
        outs.append(np.ascontiguousarray(o.transpose(1, 0, 2)).reshape(BC, L, D))
    return np.concatenate(outs, axis=0)


def kernel(sequence, token_table, genre_table, pos_table, token_genre_ids,
           genre_counts):
    nc = _get_nc()
    in_maps = prep_host_inputs(sequence, token_table, genre_table, pos_table,
                               token_genre_ids, genre_counts)
    res = run_bass_kernel_spmd(nc, in_maps, core_ids=list(range(NCORES)))
    return postprocess(res.results)
